# revision 33
# baseline (speedup 1.0000x reference)
"""Multi-head attention (B=2, L=2048, H=16, dh=64) on 8 Trainium2 NeuronCores.

Sharding: core i = (batch b=i//4) x (head-group g=i%4, 4 heads each).
Column-parallel Wq/Wk/Wv, row-parallel Wo; each core produces a partial
(L, D) bf16 output which the host sums per batch (+ bo) to unshard.

Per-core kernel (Bass/Tile), emitted as four fused "waves" (one per
512-token q-chunk j): each wave projects the K/V k-chunks the causal mask
makes newly visible (4j..4j+3) plus the wave's Q slice, runs attention for
q-chunk j, and the previous wave's output projection.

Activations/weights are bf16 (1 PE cycle/row with no min-free-dim
constraint; inputs X/Wq/Wk/Wv/Wo halve their DMA bytes); score PSUM and the
OT accumulators stay f32. Per wave:
  QT  = (Wq_g @ X^T) + bq_g            (DG, 512)  dims on partitions
  KT8 = ((Wk_g @ X^T) + bk_g) / 8      (DG, 512)
  V   = (X @ Wv_g^T) + bv_g            (512, DG)  tokens on partitions,
                                                  ones column per head
  per head h (k-chunks paired into exact-causal-trimmed tiles: chunk c
  keeps only query columns >= (c-4j)*128, masks just the 128-col triangle):
    S[k,q]   = KT8_h[:,kc]^T @ QT_h[:,j]          TensorE
    E        = exp(S + pad_bias[k])               ScalarE (bias only when
                                                  the chunk has padding)
    E       *= causal01[d]                        DVE, 128-col diag blocks
    OT[65,:]+= [V_h | 1]^T @ E                    row 64 = softmax denom
    OT_f     = OT[0:64] * bcast(1/OT[64])         DVE reciprocal + GpSimd
                                                  partition_broadcast + DVE
  out_partial = OT_f^T @ Wo_g^T                   (L, D) bf16, drains
                                                  alternate DVE/ScalarE

Startup is DMA-critical: X lives in one [128, 8*2048] tile loaded by
token-slice (slice j=0 interleaved with the packed wqkv weights across all
three DMA queues, in first-use order); K/V are only projected for live
(non-padded) key chunks; VA ones columns come from gpsimd memsets so they
can't queue behind bulk DMAs. PSUM: 2x2-bank score slots + 2 OT slots +
2 shared proj/oproj slots; the final wave's oproj also borrows the idle
score slots to keep 4 accumulation chains in flight through the tail.
Measured ~192us/core on HW (For_i slope method), rel err ~3.9e-3 vs fp32
(bf16 activations), vs ~275us/~269us for the prior f32r version.
"""
import sys
if '/opt/trn_rl_repo' not in sys.path:
    sys.path.insert(0, '/opt/trn_rl_repo')

import numpy as np

B, L, D = 2, 2048, 1024
H, DH = 16, 64
N_CORES = 8
GROUPS = 4                # tensor-parallel head groups
HG = H // GROUPS          # 4 heads per core
DG = D // GROUPS          # 256 dims per core
QCH, KCH = 512, 128       # q (free) / k (partition) chunk sizes
NQC, NKC = L // QCH, L // KCH
NEG = -1.0e30

# tuning knobs (also part of the compile cache key via _cfg());
# defaults = best HW-measured config
PW = 2          # k-chunks per scores/exp pair tile
SC_BUFS = 2     # [128, PW*512] PSUM slots, scores only
OT_BUFS = 2     # [65, 512] PSUM slots for PV accumulation
RB_OWN = False  # (unused since norm moved off PE/PSUM)
PP_OWN = 2      # [128,512] PSUM slots shared by QK/V proj + oproj
BF16_IN = True  # load X and Wq/Wk/Wv as bf16 (halves input DMA bytes)
ES_BUFS = 4     # SBUF exp-tile buffers
HEAD_IL = False  # interleave two heads' score/exp/PV chains
QK_DVE = True   # QK-proj PSUM->SBUF copies on DVE instead of ACT
OB_ACT = 0       # 0: alternate ob copies DVE/ACT, 1: all ACT, 2: all DVE
SRESET = False   # staggered semaphore reset on the repeat loop back-edge

def _cfg():
    return (PW, SC_BUFS, OT_BUFS, RB_OWN, PP_OWN, BF16_IN, ES_BUFS, HEAD_IL,
            QK_DVE, OB_ACT, SRESET)

_CACHE = {}
_RUNNERS = {}


def _emit(nc, tc, live, kbz):
    import concourse.mybir as mybir
    from concourse.bass import ts

    f32 = mybir.dt.float32
    f32r = mybir.dt.float32r
    bf16 = mybir.dt.bfloat16
    ind = bf16 if BF16_IN else f32r
    AF = mybir.ActivationFunctionType
    ALU = mybir.AluOpType
    NDC = D // KCH  # 8 contraction chunks for the projections

    t_d = {t.name: t for t in nc.m_dram_tensors()}
    xt_d, wqkv_d, wot_d = (t_d[n] for n in ("xt", "wqkv", "wot"))
    bq_d, bk8_d, bvb_d, kb_d, cm_d = (t_d[n] for n in
                                      ("bq", "bk8", "bvb", "kb", "cm"))
    onec_d, oner_d, out_d = t_d["onec"], t_d["oner"], t_d["out"]

    with (
        tc.tile_pool(name="const", bufs=1) as cpool,
        tc.tile_pool(name="big", bufs=1) as bpool,
        tc.tile_pool(name="es", bufs=ES_BUFS) as espool,
        tc.tile_pool(name="rcp", bufs=2) as rpool,
        tc.tile_pool(name="ostg", bufs=2) as opool,
        tc.tile_pool(name="psc", bufs=SC_BUFS, space="PSUM") as psc,
        tc.tile_pool(name="pot", bufs=OT_BUFS, space="PSUM") as pot,
        tc.tile_pool(name="ppp", bufs=max(PP_OWN, 1), space="PSUM") as ppp,
    ):
        projp = ppp if PP_OWN else psc
        projtag = "pp" if PP_OWN else "sc"
        # ---- constants / weights into SBUF ----
        # Wave-0 can start once wq/wk/wv + the first 512-token slice of X
        # land, so: weights first, then X split per (c, j) token slice in
        # wave order; wot (only needed at the end of wave 0) after the j=0
        # X slices.
        dma_engines = [nc.sync, nc.scalar, nc.gpsimd]
        dmi = 0

        def dma_rr(dst, src):
            nonlocal dmi
            dma_engines[dmi % 3].dma_start(dst, src)
            dmi += 1

        # X lives in one [128, NDC*L] tile; strided DMAs move several
        # contraction chunks of one token slice at a time.
        xt_all = cpool.tile([KCH, NDC * L], ind, tag="xt", name="xt")
        xt = [xt_all[:, c * L:(c + 1) * L] for c in range(NDC)]
        xt_src = xt_d.rearrange("(c p) l -> p c l", p=KCH)
        xt_dst = xt_all[:].rearrange("p (c l) -> p c l", c=NDC)
        # wqkv packs Wq|Wk|Wv column blocks -> one [128, 3*DG] DMA per c.
        # Wave-0's critical bytes (X token-slice 0 + wqkv) are split across
        # all three DMA queues, X first, so the first projection chain can
        # start after ~1/3 of the slice lands.
        wqkv_t = [cpool.tile([KCH, 3 * DG], ind, tag=f"wqkv{c}",
                             name=f"wqkv{c}") for c in range(NDC)]
        wq = [t[:, 0:DG] for t in wqkv_t]
        wk = [t[:, DG:2 * DG] for t in wqkv_t]
        wv = [t[:, 2 * DG:3 * DG] for t in wqkv_t]
        for qi, cs in enumerate(((0, 3), (3, 6), (6, 8))):
            eng = dma_engines[qi]
            for c in range(cs[0], cs[1]):
                eng.dma_start(xt_dst[:, c:c + 1, ts(0, QCH)],
                              xt_src[:, c:c + 1, ts(0, QCH)])
                eng.dma_start(wqkv_t[c][:], wqkv_d[ts(c, KCH), :])
        bq_t = cpool.tile([KCH, 2], f32, tag="bq")
        nc.sync.dma_start(bq_t[:], bq_d.rearrange("(m p) -> p m", p=KCH))
        bk_t = cpool.tile([KCH, 2], f32, tag="bk")
        nc.scalar.dma_start(bk_t[:], bk8_d.rearrange("(m p) -> p m", p=KCH))
        bvb_t = cpool.tile([KCH, DG], f32, tag="bvb")
        nc.gpsimd.dma_start(bvb_t[:], bvb_d[:])
        kb_t = cpool.tile([KCH, NKC], f32, tag="kb")
        nc.sync.dma_start(kb_t[:], kb_d[:])
        cm_t = cpool.tile([KCH, 4 * QCH], bf16, tag="cm")
        nc.scalar.dma_start(cm_t[:], cm_d[:])
        wot = []
        for c in range(DG // KCH):
            t = cpool.tile([KCH, D], bf16, tag=f"wot{c}", name=f"wot{c}")
            dma_rr(t[:], wot_d[ts(c, KCH), :])
            wot.append(t)
        for j in range(1, NQC):
            dma_rr(xt_dst[:, :, ts(j, QCH)], xt_src[:, :, ts(j, QCH)])

        # ones columns of VA (V-value independent, written once at init;
        # gpsimd memset, not DMA, so it can't queue behind the X transfers)
        def _va_ones_init():
            for t_i in live:
                va_ones = VA[t_i][:].rearrange(
                    "p (h c) -> p h c", c=DH + 1)[:, :, DH:DH + 1]
                nc.gpsimd.memset(va_ones, 1.0)

        # ---- persistent activations (bf16: flat 1 cyc/row matmuls, no
        # >=256 free-dim constraint, 2x DVE modes on the mask mults) ----
        QT = [bpool.tile([KCH, L], bf16, tag=f"qt{m}", name=f"qt{m}")
              for m in range(2)]
        KT = [bpool.tile([KCH, L], bf16, tag=f"kt{m}", name=f"kt{m}")
              for m in range(2)]
        VA = {c: bpool.tile([KCH, HG * (DH + 1)], bf16, tag=f"va{c}",
                            name=f"va{c}") for c in live}
        OTF = [bpool.tile([KCH, L], bf16, tag=f"otf{m}", name=f"otf{m}")
               for m in range(2)]
        _va_ones_init()

        # ---- fused waves: for each q-chunk j, project the K/V chunks
        # it needs (4j..4j+3), its Q slice, run attention, then O-proj.
        # This overlaps the DMA/projection ramp with ACT's exp work.
        def norm_head(j, h, ot_ps):
            hi, ho = h // 2, (h % 2) * DH
            rc = rpool.tile([1, QCH], f32, tag="rc", name="rc")
            nc.vector.reciprocal(rc[:], ot_ps[DH:DH + 1, :])
            rb = rpool.tile([DH, QCH], f32, tag="rb_s", name="rb")
            nc.gpsimd.partition_broadcast(rb[:], rc[:])
            nc.vector.tensor_tensor(
                OTF[hi][ho:ho + DH, ts(j, QCH)], ot_ps[0:DH, :], rb[:],
                op=ALU.mult)

        def oproj_wave(j, extra_pools=False):
            # the final wave's oproj has nothing left to overlap with, so
            # borrow the (now idle) scores PSUM slots to keep 4 accumulation
            # chains in flight instead of 2
            opi = 0
            for t_i in range(4 * j, 4 * j + 4):
                ob = opool.tile([KCH, D], bf16, tag="ob", name="ob")
                for g0 in range(D // QCH):
                    if extra_pools and opi % 2:
                        op_ps = psc.tile([KCH, QCH], f32, tag="sc",
                                         name="op_ps")
                    else:
                        op_ps = projp.tile([KCH, QCH], f32, tag=projtag,
                                           name="op_ps")
                    opi += 1
                    for c in range(2):
                        nc.tensor.matmul(
                            op_ps[:],
                            OTF[c][:, ts(t_i, KCH)],
                            wot[c][:, ts(g0, QCH)],
                            start=(c == 0), stop=(c == 1))
                    # alternate drains across DVE and ACT so the final
                    # wave keeps both engines + both DMA queues busy
                    if g0 % 2 == 0:
                        nc.vector.tensor_copy(ob[:, ts(g0, QCH)], op_ps[:])
                    else:
                        nc.scalar.activation(ob[:, ts(g0, QCH)], op_ps[:],
                                             AF.Copy)
                dma_engines[t_i % 3].dma_start(out_d[ts(t_i, KCH), :], ob[:])

        # K (and V) are only consumed for live key chunks; queries at padded
        # positions still produce output, so Q stays full width.
        klive = (max(live) + 1) * KCH

        def proj_qk(j, m):
            for dst, w_l, b_t, scale, w in (
                    (QT, wq, bq_t, 1.0, QCH),
                    (KT, wk, bk_t, 0.125,
                     max(0, min(QCH, klive - j * QCH)))):
                if w == 0:
                    continue
                ps = projp.tile([KCH, QCH], f32, tag=projtag, name="ps")
                for c in range(NDC):
                    nc.tensor.matmul(
                        ps[:, 0:w], w_l[c][:, ts(m, KCH)],
                        xt[c][:, j * QCH:j * QCH + w],
                        start=(c == 0), stop=(c == NDC - 1))
                if QK_DVE:
                    nc.vector.tensor_scalar(
                        dst[m][:, j * QCH:j * QCH + w], ps[:, 0:w], scale,
                        b_t[:, m:m + 1], op0=ALU.mult, op1=ALU.add)
                else:
                    nc.scalar.activation(dst[m][:, j * QCH:j * QCH + w],
                                         ps[:, 0:w],
                                         AF.Identity,
                                         bias=b_t[:, m:m + 1],
                                         scale=scale)

        for j in range(NQC):
            # m=0 half of Q/K first: heads 0-1 depend only on it, so
            # their attention starts while the m=1 half still projects.
            proj_qk(j, 0)
            # V projection for this wave's 4 token chunks (live only)
            for t_i in range(4 * j, 4 * j + 4):
                if t_i not in live:
                    continue
                ps = projp.tile([KCH, DG], f32, tag=projtag, name="vps")
                for c in range(NDC):
                    nc.tensor.matmul(ps[:], xt[c][:, ts(t_i, KCH)], wv[c][:],
                                     start=(c == 0), stop=(c == NDC - 1))
                nc.vector.tensor_tensor(
                    VA[t_i][:].rearrange("p (h c) -> p h c",
                                         c=DH + 1)[:, :, 0:DH],
                    ps[:].rearrange("p (h c) -> p h c", c=DH),
                    bvb_t[:].rearrange("p (h c) -> p h c", c=DH),
                    op=ALU.add)

            # attention for q-chunk j; k-chunks paired into [128, 1024]
            # tiles so exp/mask run at half the op count.
            livec = [c for c in live if c * KCH <= j * QCH + QCH - 1]
            pairs = [livec[i:i + PW] for i in range(0, len(livec), PW)]

            prev_norm = None

            def emit_head_group(hs):
                nonlocal prev_norm
                ots, mkers = {}, {}
                for h in hs:
                    hi, ho = h // 2, (h % 2) * DH
                    ot_ps = pot.tile([DH + 1, QCH], f32, tag="ot",
                                     name="ot_ps")
                    ots[h] = ot_ps

                    def make(h, hi, ho, ot_ps):
                        def chunk_lay(pr):
                            # exact causal start per chunk: chunk c's first
                            # visible query column is (c-4j)*KCH (bf16
                            # matmuls have no min-free-dim constraint)
                            q0s = [max(0, (c - 4 * j) * KCH) for c in pr]
                            qns = [QCH - q0 for q0 in q0s]
                            offs = [sum(qns[:i]) for i in range(len(pr) + 1)]
                            return q0s, offs

                        def score_pair(pr):
                            q0s, offs = chunk_lay(pr)
                            w = offs[-1]
                            s_ps = psc.tile([KCH, PW * QCH], f32, tag="sc",
                                            name="s_ps")
                            for i, c in enumerate(pr):
                                nc.tensor.matmul(
                                    s_ps[:, offs[i]:offs[i + 1]],
                                    KT[hi][ho:ho + DH, ts(c, KCH)],
                                    QT[hi][ho:ho + DH,
                                           j * QCH + q0s[i]:(j + 1) * QCH],
                                    start=True, stop=True)
                            es = espool.tile([KCH, PW * QCH], bf16, tag="es",
                                             name="es")
                            if all(c in kbz for c in pr):
                                nc.scalar.activation(
                                    es[:, :w], s_ps[:, :w], AF.Exp)
                            else:
                                for i, c in enumerate(pr):
                                    nc.scalar.activation(
                                        es[:, offs[i]:offs[i + 1]],
                                        s_ps[:, offs[i]:offs[i + 1]],
                                        AF.Exp, bias=kb_t[:, c:c + 1])
                            for i, c in enumerate(pr):
                                d = c - 4 * j
                                if d < 0:
                                    continue
                                # partial triangle spans only the first KCH
                                # columns past q0; beyond that the chunk is
                                # fully visible
                                nc.vector.tensor_tensor(
                                    es[:, offs[i]:offs[i] + KCH],
                                    es[:, offs[i]:offs[i] + KCH],
                                    cm_t[:, d * QCH + q0s[i]:
                                         d * QCH + q0s[i] + KCH],
                                    op=ALU.mult)
                            return es

                        def pv_pair(pi, es):
                            pr = pairs[pi]
                            q0s, offs = chunk_lay(pr)
                            for i, c in enumerate(pr):
                                nc.tensor.matmul(
                                    ot_ps[:, q0s[i]:QCH],
                                    VA[c][:, ts(h, DH + 1)],
                                    es[:, offs[i]:offs[i + 1]],
                                    start=(pi == 0 and i == 0),
                                    stop=(pi == len(pairs) - 1
                                          and i == len(pr) - 1))
                        return score_pair, pv_pair
                    mkers[h] = make(h, hi, ho, ot_ps)

                es_q = {h: [mkers[h][0](pairs[0])] for h in hs}
                if len(pairs) > 1:
                    for h in hs:
                        es_q[h].append(mkers[h][0](pairs[1]))
                if prev_norm is not None:
                    for pn in prev_norm:
                        norm_head(*pn)
                    prev_norm = None
                for pi in range(len(pairs)):
                    for h in hs:
                        if pi + 2 < len(pairs):
                            es_q[h].append(mkers[h][0](pairs[pi + 2]))
                        mkers[h][1](pi, es_q[h][pi])
                prev_norm = [(j, h, ots[h]) for h in hs]

            if j > 0:
                oproj_wave(j - 1)
            if HEAD_IL:
                emit_head_group((0, 1))
                proj_qk(j, 1)
                emit_head_group((2, 3))
            else:
                emit_head_group((0,))
                emit_head_group((1,))
                proj_qk(j, 1)
                emit_head_group((2,))
                emit_head_group((3,))
            for pn in prev_norm:
                norm_head(*pn)
        oproj_wave(NQC - 1, extra_pools=True)


def _build(live, kbz, repeat=1):
    """Compile the SPMD Bass program. `live` = k-chunks not fully key-padded
    on every core. repeat > 1 wraps the body in a HW loop (timing only)."""
    import concourse.bacc as bacc
    import concourse.tile as tile
    import concourse.mybir as mybir

    f32 = mybir.dt.float32
    f32r = mybir.dt.float32r

    nc = bacc.Bacc("TRN2", target_bir_lowering=False, debug=False,
                   num_devices=N_CORES)
    bf16 = mybir.dt.bfloat16
    ind = bf16 if BF16_IN else f32r
    dts = []
    dts.append(nc.dram_tensor("xt", [D, L], ind, kind="ExternalInput"))
    dts.append(nc.dram_tensor("wqkv", [D, 3 * DG], ind,
                              kind="ExternalInput"))
    dts.append(nc.dram_tensor("wot", [DG, D], bf16, kind="ExternalInput"))
    dts.append(nc.dram_tensor("bq", [DG], f32, kind="ExternalInput"))
    dts.append(nc.dram_tensor("bk8", [DG], f32, kind="ExternalInput"))
    dts.append(nc.dram_tensor("bvb", [KCH, DG], f32, kind="ExternalInput"))
    dts.append(nc.dram_tensor("kb", [KCH, NKC], f32, kind="ExternalInput"))
    dts.append(nc.dram_tensor("cm", [KCH, 4 * QCH], bf16,
                              kind="ExternalInput"))
    dts.append(nc.dram_tensor("onec", [KCH, HG], bf16, kind="ExternalInput"))
    dts.append(nc.dram_tensor("oner", [1, DH], f32r, kind="ExternalInput"))
    dts.append(nc.dram_tensor("out", [L, D], bf16, kind="ExternalOutput"))
    nc.m_dram_tensors = lambda: dts

    with tile.TileContext(nc) as tc:
        if repeat > 1:
            # the unrolled body far exceeds one 16KiB IRAM block per engine,
            # so arm branch-prefetch hints to avoid ~3-4us back-edge ifetch
            # stalls on every iteration
            with tc.For_i(0, repeat, 1, staggered_reset=SRESET):
                _emit(nc, tc, live, kbz)
        else:
            _emit(nc, tc, live, kbz)

    nc.compile()
    return nc


def _prep_inputs(X, Wq, bq, Wk, bk, Wv, bv, Wo, bo, key_padding_mask):
    """Host-side sharding: per-core input dicts + the live k-chunk list."""
    mask = np.asarray(key_padding_mask)
    dead = [bool(mask[:, c * KCH:(c + 1) * KCH].all()) for c in range(NKC)]
    live = tuple(c for c in range(NKC) if not dead[c])
    kbz = frozenset(c for c in live
                    if not mask[:, c * KCH:(c + 1) * KCH].any())

    import ml_dtypes
    bf16 = ml_dtypes.bfloat16
    kk = np.arange(KCH, dtype=np.float32)[:, None]
    qq = np.arange(QCH, dtype=np.float32)[None, :]
    cm = np.concatenate(
        [(128 * d + kk <= qq).astype(bf16) for d in range(4)], axis=1)

    ind = bf16 if BF16_IN else np.float32
    in_maps = []
    for core in range(N_CORES):
        b, g = core // GROUPS, core % GROUPS
        gs = slice(DG * g, DG * (g + 1))
        kb = np.where(mask[b], np.float32(NEG), np.float32(0.0))
        in_maps.append({
            "xt": np.ascontiguousarray(X[b].T).astype(ind),
            "wqkv": np.ascontiguousarray(np.concatenate(
                [Wq[gs, :].T, Wk[gs, :].T, Wv[gs, :].T], axis=1)).astype(ind),
            "wot": np.ascontiguousarray(Wo[:, gs].T).astype(bf16),
            "bq": np.ascontiguousarray(bq[gs], dtype=np.float32),
            "bk8": np.ascontiguousarray(bk[gs] / 8.0, dtype=np.float32),
            "bvb": np.broadcast_to(
                bv[gs].astype(np.float32), (KCH, DG)).copy(),
            "kb": np.ascontiguousarray(
                kb.reshape(NKC, KCH).T, dtype=np.float32),
            "cm": cm,
            "onec": np.ones((KCH, HG), dtype=bf16),
            "oner": np.ones((1, DH), dtype=np.float32),
        })
    return in_maps, live, kbz


def _get_compiled(live, kbz, repeat=1):
    key = (live, kbz, repeat, _cfg())
    if key not in _CACHE:
        _CACHE[key] = _build(live, kbz, repeat)
    return _CACHE[key]


class _Runner:
    """Persistent jitted SPMD executable (mirrors bass2jax.run_bass_via_pjrt
    but keeps the compiled callable so repeated runs skip jit/NEFF reload)."""

    def __init__(self, nc, donate=True):
        import jax
        import numpy as _np
        from jax.sharding import Mesh, PartitionSpec
        from jax.experimental.shard_map import shard_map
        import concourse.mybir as mybir
        from concourse.bass2jax import (
            install_neuronx_cc_hook, _bass_exec_p, partition_id_tensor)

        install_neuronx_cc_hook()
        part_name = (nc.partition_id_tensor.name
                     if nc.partition_id_tensor else None)
        in_names, out_names, out_avals = [], [], []
        for alloc in nc.m.functions[0].allocations:
            if not isinstance(alloc, mybir.MemoryLocationSet):
                continue
            name = alloc.memorylocations[0].name
            if alloc.kind == "ExternalInput":
                if name != part_name:
                    in_names.append(name)
            elif alloc.kind == "ExternalOutput":
                out_names.append(name)
                out_avals.append(jax.core.ShapedArray(
                    tuple(alloc.tensor_shape), mybir.dt.np(alloc.dtype)))
        self.in_names, self.out_names, self.out_avals = \
            in_names, out_names, out_avals
        n_params, n_outs = len(in_names), len(out_avals)
        all_names = list(in_names + out_names)
        if part_name is not None:
            all_names.append(part_name)
        all_names = tuple(all_names)
        avals = tuple(out_avals)

        def _body(*args):
            operands = list(args)
            if part_name is not None:
                operands.append(partition_id_tensor())
            return tuple(_bass_exec_p.bind(
                *operands, out_avals=avals, in_names=all_names,
                out_names=tuple(out_names),
                lowering_input_output_aliases=(),
                sim_require_finite=True, sim_require_nnan=True, nc=nc))

        devices = jax.devices()[:N_CORES]
        self.mesh = Mesh(_np.asarray(devices), ("core",))
        wrapped = shard_map(
            _body, mesh=self.mesh,
            in_specs=(PartitionSpec("core"),) * (n_params + n_outs),
            out_specs=(PartitionSpec("core"),) * n_outs, check_rep=False)
        donate_args = tuple(range(n_params, n_params + n_outs)) if donate \
            else ()
        self._fn = jax.jit(wrapped, donate_argnums=donate_args,
                           keep_unused=True)
        self._zero_shapes = [
            ((N_CORES * a.shape[0],) + tuple(a.shape[1:]), a.dtype)
            for a in out_avals]

    def concat_inputs(self, in_maps):
        return [
            np.concatenate([np.asarray(m[name]) for m in in_maps], axis=0)
            for name in self.in_names]

    def __call__(self, in_maps):
        import jax
        concat_in = self.concat_inputs(in_maps)
        zeros = [np.zeros(s, d) for s, d in self._zero_shapes]
        out = self._fn(*concat_in, *zeros)
        out = jax.block_until_ready(out)
        return [
            {name: np.asarray(out[i]).reshape(
                N_CORES, *self.out_avals[i].shape)[c]
             for i, name in enumerate(self.out_names)}
            for c in range(N_CORES)]

    def timed(self, in_maps, iters=20):
        """Per-call wall times with device-resident inputs, no host readback.
        Use with donate=False so buffers survive across calls."""
        import time
        import jax
        from jax.sharding import NamedSharding, PartitionSpec
        sh = NamedSharding(self.mesh, PartitionSpec("core"))
        dev_in = [jax.device_put(a, sh) for a in self.concat_inputs(in_maps)]
        dev_zeros = [jax.device_put(np.zeros(s, d), sh)
                     for s, d in self._zero_shapes]
        jax.block_until_ready(dev_in)
        jax.block_until_ready(dev_zeros)
        times = []
        for _ in range(iters):
            t0 = time.perf_counter()
            out = self._fn(*dev_in, *dev_zeros)
            jax.block_until_ready(out)
            times.append(time.perf_counter() - t0)
        return np.array(times)


def _get_runner(live, kbz, repeat=1, donate=True):
    key = (live, kbz, repeat, donate, _cfg())
    if key not in _RUNNERS:
        _RUNNERS[key] = _Runner(_get_compiled(live, kbz, repeat),
                                donate=donate)
    return _RUNNERS[key]


def kernel(X, Wq, bq, Wk, bk, Wv, bv, Wo, bo, key_padding_mask):
    from concourse.bass_utils import run_bass_kernel_spmd

    in_maps, live, kbz = _prep_inputs(X, Wq, bq, Wk, bk, Wv, bv, Wo, bo,
                                      key_padding_mask)
    nc = _get_compiled(live, kbz)
    res = run_bass_kernel_spmd(nc, in_maps, list(range(N_CORES)))
    out = np.zeros((B, L, D), dtype=np.float32)
    for core in range(N_CORES):
        out[core // GROUPS] += res.results[core]["out"].astype(np.float32)
    out += np.asarray(bo, dtype=np.float32)[None, None, :]
    return out

